# revision 1
# baseline (speedup 1.0000x reference)
"""ConvQRNN Trainium2 kernel (v2).

Strategy (8 NeuronCores, spatial H-sharding, 8 rows/core):
  - Conv3d(k=(2,3,3), CIN=3 -> 256) lowered to matmul: host builds a fp16
    im2col with K=56 rows (54 taps + ones row carrying the conv bias + one
    zero pad row).  Per timestep each gate's [64ch x 2048pix] pre-activation
    is computed as 16 matmuls (gate x partition-half x batch-stream) into a
    single statically-placed fp32 PSUM tile (8 banks, bank-aligned slices).
  - Peephole adds (a_if = E_if + Wc*C) are done ON THE TENSOR ENGINE via
    identity-matmul accumulation (start=False) into the same PSUM banks,
    keeping them off the DVE.
  - Scan layout: [128, *] fp16 with partition = (b//2)*64 + ch; free is
    organized per batch-stream q = b%2 (512 hw pixels each).  Two streams
    are kept dependency-separate through the sigmoid/cell chain so their
    per-step critical paths pipeline across engines.
  - o-gate tail (sigmoid, tanh(C), H=og*tanh(C)) is batched over KAPPA=8
    timesteps; the H multiply runs on GPSIMD to keep DVE lean.
"""

import os

import numpy as np

B, CIN, T, H, W = 4, 3, 32, 64, 64
COUT = 64
NC = 8
HS = H // NC
K = 56  # 54 conv taps + ones(bias) row + zero pad row
PIX = B * HS * W          # 2048 pixels per core per timestep
HWp = HS * W              # 512 pixels per stream (hw)
KAPPA = 8                 # o-gate tail batch (timesteps)
NWIN = T // KAPPA

f16 = np.float16

_CACHE = {}

LAST_RESULTS = {}


def _host_prep(X, Wconv, bconv, W_ci, W_cf, W_co):
    X = np.ascontiguousarray(np.asarray(X, np.float32))
    Wconv = np.asarray(Wconv, np.float32)
    bconv = np.asarray(bconv, np.float32)
    Xp = np.pad(X, ((0, 0), (0, 0), (1, 0), (1, 1), (1, 1)))  # (B,CIN,T+1,H+2,W+2)

    im2col = np.zeros((NC, K, T, PIX), f16)
    for c in range(NC):
        for cin in range(CIN):
            for dt in range(2):
                for dh in range(3):
                    for dw in range(3):
                        k = ((cin * 2 + dt) * 3 + dh) * 3 + dw
                        blk = Xp[:, cin, dt:dt + T,
                                 8 * c + dh:8 * c + dh + HS, dw:dw + W]
                        blk = blk.reshape(2, 2, T, HS, W).transpose(2, 0, 1, 3, 4)
                        im2col[c, k] = blk.reshape(T, PIX).astype(f16)
        im2col[c, 54] = 1.0

    lhsT = np.zeros((4, K, 128), f16)
    Wr = Wconv.reshape(4, COUT, CIN, 2, 3, 3)
    for g in range(4):
        wk = Wr[g].transpose(1, 2, 3, 4, 0).reshape(54, COUT).astype(f16)
        lhsT[g, :54, :64] = wk
        lhsT[g, :54, 64:] = wk
        lhsT[g, 54, :64] = bconv[g * 64:(g + 1) * 64].astype(f16)
        lhsT[g, 54, 64:] = bconv[g * 64:(g + 1) * 64].astype(f16)

    # peephole weights:
    #   wcif [NC, 128, 2048] free = (q, {i,f}, hw)
    #   wco  [NC, 128, 1024] free = (q, hw)
    wcif = np.zeros((NC, 128, 2 * PIX // 2), f16)
    wco = np.zeros((NC, 128, PIX // 2), f16)
    Wci = np.asarray(W_ci, np.float32)
    Wcf = np.asarray(W_cf, np.float32)
    Wco_ = np.asarray(W_co, np.float32)
    for c in range(NC):
        ci = Wci[:, 8 * c:8 * c + HS, :].reshape(64, HWp).astype(f16)
        cf = Wcf[:, 8 * c:8 * c + HS, :].reshape(64, HWp).astype(f16)
        co = Wco_[:, 8 * c:8 * c + HS, :].reshape(64, HWp).astype(f16)
        for half in range(2):
            rows = slice(64 * half, 64 * half + 64)
            for q in range(2):
                wcif[c, rows, q * 1024:q * 1024 + 512] = ci
                wcif[c, rows, q * 1024 + 512:q * 1024 + 1024] = cf
                wco[c, rows, q * 512:q * 512 + 512] = co
    ident = np.eye(128, dtype=f16)
    return im2col, lhsT, wcif, wco, ident


def _build_nc(loop_reps=1):
    import concourse.bacc as bacc
    import concourse.mybir as mybir
    from contextlib import nullcontext
    from concourse.tile import TileContext

    fp16 = mybir.dt.float16
    fp32 = mybir.dt.float32
    AF = mybir.ActivationFunctionType

    nc = bacc.Bacc(None, target_bir_lowering=False)

    im2col_d = nc.dram_tensor("im2col", [K, T, PIX], fp16, kind="ExternalInput")
    lhsT_d = nc.dram_tensor("lhsT", [4, K, 128], fp16, kind="ExternalInput")
    wcif_d = nc.dram_tensor("wcif", [128, 2048], fp16, kind="ExternalInput")
    wco_d = nc.dram_tensor("wco", [128, 1024], fp16, kind="ExternalInput")
    ident_d = nc.dram_tensor("ident", [128, 128], fp16, kind="ExternalInput")
    out_d = nc.dram_tensor("out", [NWIN, 128, KAPPA * 1024], fp16,
                           kind="ExternalOutput")

    with TileContext(nc) as tc:
        with (
            tc.tile_pool(name="const", bufs=1) as constp,
            tc.tile_pool(name="state", bufs=1) as statep,
            tc.tile_pool(name="rhs", bufs=3) as rhsp,
            tc.tile_pool(name="vif", bufs=2) as vifp,
            tc.tile_pool(name="sif", bufs=2) as sifp,
            tc.tile_pool(name="p12", bufs=2) as p12p,
            tc.tile_pool(name="vo", bufs=2) as vop,
            tc.tile_pool(name="tail", bufs=2) as tailp,
            tc.tile_pool(name="psum", bufs=1, space="PSUM") as psump,
        ):
            wcif = constp.tile([128, 2048], fp16)
            wco = constp.tile([128, 1024], fp16)
            id_t = constp.tile([128, 128], fp16)
            lhsT_sb = constp.tile([K, 4 * 128], fp16)
            nc.sync.dma_start(out=wcif[:], in_=wcif_d[:])
            nc.sync.dma_start(out=wco[:], in_=wco_d[:])
            nc.sync.dma_start(out=id_t[:], in_=ident_d[:])
            nc.sync.dma_start(
                out=lhsT_sb[:].rearrange("k (g m) -> k g m", g=4),
                in_=lhsT_d[:].rearrange("g k m -> k g m"),
            )

            # tgC ring: [128, q(2), slot(8), (tg 512 | C 512)]
            # slot s holds [tanh(g_s) | C_{s-1}]
            tgC = statep.tile([128, 2, KAPPA, 1024], fp16)
            # ao ring: [128, slot(8), (q, hw)] with slot = t % 8
            ao = statep.tile([128, KAPPA, 1024], fp16)
            # h ring for one window of outputs
            hr = statep.tile([128, KAPPA, 1024], fp16)
            nc.vector.memset(tgC[:, :, 0, 512:1024], 0.0)

            # PSUM, bank-aligned slices of one static tile [128, 4096] fp32:
            #   [0:1024]    eif_A (i_A [0:512], f_A [512:1024])   banks 0-1
            #   [1024:2048] eif_B                                  banks 2-3
            #   [2048:3072] eg  (q, hw)                            banks 4-5
            #   [3072:4096] eo  (q, hw)                            banks 6-7
            psum = psump.tile([128, 4096], fp32)

            GATE = {  # psum free offset per (gate, q); i/f groups end at ident
                "i": (0, 1024, False),
                "f": (512, 1024, False),
                "g": (2048, 512, True),
                "o": (3072, 512, True),
            }

            def emit_conv(t, rhs_t, gates):
                for gname in gates:
                    gi = "ifgo".index(gname)
                    base, qstride, stop = GATE[gname]
                    for q in range(2):
                        off = base + q * qstride
                        for hf in range(2):
                            lw = lhsT_sb[:, gi * 128 + 64 * hf:
                                         gi * 128 + 64 * hf + 64]
                            nc.tensor.matmul(
                                psum[64 * hf:64 * hf + 64, off:off + 512],
                                lw,
                                rhs_t[:, 1024 * hf + 512 * q:
                                      1024 * hf + 512 * q + 512],
                                start=True,
                                stop=stop,
                                tile_position=(0, 64 * hf),
                            )

            def emit_ao(t):
                # a_o(t) = e_o + v_o(t) -> ao slot t%8 (deferred to body t+1
                # so next step's v_if isn't queued behind the Pool v_o wait)
                nc.vector.tensor_add(
                    out=ao[:, t % KAPPA, :], in0=psum[:, 3072:4096],
                    in1=vo_tiles.pop(t)[:])

            def emit_tail_one(t0):
                # sigmoid(a_o)/tanh(C)/H for step t0 (fills ACT gap each step)
                j = t0 % KAPPA
                s_o1 = tailp.tile([128, 1024], fp16)
                nc.scalar.activation(s_o1[:], ao[:, j, :], AF.Sigmoid)
                t_c1 = tailp.tile([128, 1024], fp16)
                nc.scalar.activation(
                    t_c1[:].rearrange("p (q n) -> p q n", q=2),
                    tgC[:, :, (t0 + 1) % KAPPA, 512:1024], AF.Tanh)
                nc.gpsimd.tensor_mul(out=hr[:, j, :], in0=s_o1[:], in1=t_c1[:])
                if j + 1 == KAPPA:
                    nc.sync.dma_start(out=out_d[t0 // KAPPA],
                                      in_=hr[:].rearrange("p s n -> p (s n)"))

            loop_cm = tc.For_i(0, loop_reps) if loop_reps > 1 else nullcontext()
            with loop_cm:
                rhs_tiles = {}
                vo_tiles = {}
                for t in range(2):
                    rhs_tiles[t] = rhsp.tile([K, PIX], fp16, name="rhs")
                    nc.sync.dma_start(out=rhs_tiles[t][:], in_=im2col_d[:, t, :])
                emit_conv(0, rhs_tiles[0], "ifgo")

                for t in range(T):
                    s = t % KAPPA
                    s1 = (t + 1) % KAPPA
                    if t + 2 < T:
                        rhs_tiles[t + 2] = rhsp.tile([K, PIX], fp16, name="rhs")
                        nc.sync.dma_start(out=rhs_tiles[t + 2][:],
                                          in_=im2col_d[:, t + 2, :])

                    c_prev = tgC[:, :, s, 512:1024]        # [128, 2, 512]
                    # per-stream: v_if -> identity-add -> sigmoid
                    v_if = vifp.tile([128, 2048], fp16)
                    s_if = {}
                    for q in range(2):
                        cp = c_prev[:, q:q + 1, :].broadcast_to([128, 2, 512])
                        nc.vector.tensor_mul(
                            out=v_if[:, q * 1024:(q + 1) * 1024].rearrange(
                                "p (r n) -> p r n", r=2),
                            in0=wcif[:, q * 1024:(q + 1) * 1024].rearrange(
                                "p (r n) -> p r n", r=2),
                            in1=cp,
                        )
                        for g in range(2):
                            off = q * 1024 + g * 512
                            nc.tensor.matmul(
                                psum[:, off:off + 512],
                                id_t[:],
                                v_if[:, off:off + 512],
                                start=False,
                                stop=True,
                            )
                        s_if[q] = sifp.tile([128, 1024], fp16, name=f"sif{q}")
                        nc.scalar.activation(
                            s_if[q][:], psum[:, q * 1024:(q + 1) * 1024],
                            AF.Sigmoid)

                    # next step's i/f conv right behind the ident-adds so the
                    # sigmoid->conv->ident->sigmoid PE chain stays short
                    if t + 1 < T:
                        emit_conv(t + 1, rhs_tiles[t + 1], "if")
                    # deferred a_o of previous step, then conv_o(t) which
                    # reuses the e_o banks
                    if t > 0:
                        emit_ao(t - 1)
                        emit_conv(t, rhs_tiles.pop(t), "o")

                    # tanh(g) for both streams -> tg halves of slot s
                    nc.scalar.activation(
                        tgC[:, :, s, 0:512], psum[:, 2048:3072].rearrange(
                            "p (q n) -> p q n", q=2), AF.Tanh)

                    p12 = p12p.tile([128, 2, 2, 512], fp16)  # (q, {p1,p2}, hw)
                    for q in range(2):
                        nc.vector.tensor_mul(
                            out=p12[:, q], in0=s_if[q][:].rearrange(
                                "p (r n) -> p r n", r=2),
                            in1=tgC[:, q, s, :].rearrange(
                                "p (r n) -> p r n", r=2),
                        )
                        nc.vector.tensor_add(
                            out=tgC[:, q, s1, 512:1024],
                            in0=p12[:, q, 0], in1=p12[:, q, 1],
                        )
                    if t + 1 < T:
                        emit_conv(t + 1, rhs_tiles[t + 1], "g")

                    # v_o on GPSIMD (off critical path)
                    c_new = tgC[:, :, s1, 512:1024]        # [128, 2, 512]
                    v_o = vop.tile([128, 1024], fp16)
                    nc.gpsimd.tensor_mul(
                        out=v_o[:].rearrange("p (q n) -> p q n", q=2),
                        in0=wco[:].rearrange("p (q n) -> p q n", q=2),
                        in1=c_new,
                    )
                    vo_tiles[t] = v_o

                    # o-gate tail for step t-1, after a_o(t-1)
                    if t >= 1:
                        emit_tail_one(t - 1)

                # epilogue: last a_o + final tail step
                emit_ao(T - 1)
                emit_tail_one(T - 1)

    nc.compile()
    return nc


def _get_nc():
    if "nc" not in _CACHE:
        _CACHE["nc"] = _build_nc()
    return _CACHE["nc"]


OUT_NAMES = ["out"]


def _core_inputs(prep, c):
    im2col, lhsT, wcif, wco, ident = prep
    return {"im2col": im2col[c], "lhsT": lhsT, "wcif": wcif[c],
            "wco": wco[c], "ident": ident}


def _unshard_one(outs, c):
    # out [NWIN, 128, KAPPA*1024]; free = (slot, q, hw); slot = t % 8
    o = np.asarray(outs["out"], f16).astype(np.float32)
    o = o.reshape(NWIN, 2, 64, KAPPA, 2, HS, W)  # (w, b_hi, ch, s, q, h, w)
    o = o.transpose(1, 4, 2, 0, 3, 5, 6)  # (b_hi, q, ch, w, s, h, w)
    return o.reshape(B, COUT, T, HS, W)


def _ref_shard(expected, c):
    return expected[:, :, :, 8 * c:8 * c + HS, :]


def kernel(X, Wconv, bconv, W_ci, W_cf, W_co):
    from concourse.bass_utils import run_bass_kernel_spmd

    prep = _host_prep(X, Wconv, bconv, W_ci, W_cf, W_co)
    nc = _get_nc()
    in_maps = [_core_inputs(prep, c) for c in range(NC)]
    trace = bool(os.environ.get("QRNN_TRACE"))
    res = run_bass_kernel_spmd(
        nc, in_maps, core_ids=list(range(NC)), trace=trace
    )
    LAST_RESULTS["exec_time_ns"] = getattr(res, "exec_time_ns", None)

    O = np.empty((B, COUT, T, H, W), np.float32)
    for c in range(NC):
        O[:, :, :, 8 * c:8 * c + HS, :] = _unshard_one(res.results[c], c)
    return O

